# revision 9
# baseline (speedup 1.0000x reference)
"""GCN layer (GCNConv + log_softmax) on 8 Trainium2 NeuronCores.

Sharding: nodes row-sharded 8 ways. Each core computes h' = dis * (x @ W)
for its slice (fp8-e3m4 GEMM with W pre-scaled by 64, rescaled in the
Scalar-engine copy), AllGathers h' in two chunks (A: rows 0-3967 fired
mid-GEMM, B: the 2304-row tail fired at GEMM end) and aggregates
messages for its destination slice with dma_gather calls spread over 4
SWDGE queues (parallel Q7 descriptor generation) + host-built fp8
one-hot indicator matmuls on the tensor engine.  Self-loop and bias
terms are pre-folded into the per-tile partial buffer during the GEMM,
so the epilogue is only: z = acc + partial, row-max, then log_softmax
with the dis/shift folded into the Scalar activation's scale/bias
operands.
"""

import numpy as np
import ml_dtypes

import concourse.bass as bass
import concourse.tile as tile
from concourse import bacc, mybir
from concourse.bass_utils import run_bass_kernel_spmd

bf16 = ml_dtypes.bfloat16
f8 = ml_dtypes.float8_e3m4
F32 = mybir.dt.float32
BF16 = mybir.dt.bfloat16
FP8 = mybir.dt.float8e3
I16 = mybir.dt.int16

N_NODES = 50000
D_IN = 2048
D_OUT = 512
C = 8                      # cores
NLOC = N_NODES // C        # 6250 real nodes per core
T = 49                     # dst tiles per core
NPAD = T * 128             # 6272 padded rows per core
SW = 512                   # GEMM node-chunk width
NCHUNK = 13                # 12 full chunks + 128-row tail
WSCALE = 64.0              # weight pre-scale so fp8-e3m4 w stays normal
TILE_A = 31                # dst tiles in gather space A
LOCS = (TILE_A * 128, NPAD - TILE_A * 128)     # (3968, 2304) rows per core
EDGE0 = (0, TILE_A * 128)
KT = D_IN // 128           # 16 contraction chunks
DEPTH = 12                 # gather tile pool depth
NQ = 4                     # SWDGE queues (parallel Q7 descriptor gen)
EG = 7                     # epilogue group size (tiles per Ln batch)

LAST_RESULTS = None        # test harness reads exec_time_ns from here


def _wrap_idx(idx):
    """Wrap a [n] index array into the [128, n//16] dma_gather layout."""
    n = idx.shape[0]
    assert n % 16 == 0
    cols = n // 16
    w = np.empty((128, cols), np.int16)
    blk = idx.reshape(cols, 16).T.astype(np.int16)   # [16, cols]
    for g in range(8):
        w[g * 16:(g + 1) * 16, :] = blk
    return w


def _preprocess(x, edge_index, weight, bias):
    src = np.asarray(edge_index[0], dtype=np.int64)
    dst = np.asarray(edge_index[1], dtype=np.int64)

    # degree includes the self loop; self-loop messages are served from the
    # SBUF copy of h' instead of being gathered.
    deg = (np.bincount(dst, minlength=N_NODES) + 1).astype(np.float32)
    dis = 1.0 / np.sqrt(deg)

    sc = src // NLOC
    sr = src % NLOC
    half = (sr >= EDGE0[1]).astype(np.int64)         # gather space 0/1
    base = np.asarray(LOCS)[half]
    g = sc * base + (sr - np.asarray(EDGE0)[half])

    dc = dst // NLOC                   # dst core
    dr = dst % NLOC
    dt = dr // 128                     # dst tile within core
    dl = dr % 128                      # dst row within tile

    order = np.lexsort((g, half, dt, dc))
    g, dc, dt, dl, half = g[order], dc[order], dt[order], dl[order], half[order]

    key = (dc * T + dt) * 2 + half
    counts = np.bincount(key, minlength=C * T * 2).reshape(C, T, 2)
    blocks = -(-counts // 128)                       # ceil div
    NB = blocks.max(axis=0)                          # [T, 2]

    blk_cols = int(NB.sum())
    idx_cols = 8 * blk_cols
    idx_np = np.zeros((C, 128, idx_cols), np.int16)
    oh_np = np.zeros((C, 128, blk_cols * 128), f8)
    dcol = np.arange(128)

    starts = np.zeros(C * T * 2 + 1, np.int64)
    np.cumsum(np.bincount(key, minlength=C * T * 2), out=starts[1:])

    for c in range(C):
        icol = 0
        bcol = 0
        for t in range(T):
            for h in range(2):
                B = int(NB[t, h])
                if B == 0:
                    continue
                k = (c * T + t) * 2 + h
                seg = slice(starts[k], starts[k + 1])
                n = starts[k + 1] - starts[k]
                cap = B * 128
                gi = np.zeros(cap, np.int64)
                gi[:n] = g[seg]
                dv = np.full(cap, -1.0, np.float32)
                dv[:n] = dl[seg]
                idx_np[c, :, icol:icol + 8 * B] = _wrap_idx(gi)
                ohb = (dv.reshape(B, 128)[:, :, None] == dcol[None, None, :])
                oh_np[c, :, bcol * 128:(bcol + B) * 128] = (
                    ohb.transpose(1, 0, 2).reshape(128, B * 128).astype(f8))
                icol += 8 * B
                bcol += B

    w_f8 = np.ascontiguousarray((weight * WSCALE).astype(f8))
    xT = np.zeros((C, D_IN, NPAD), f8)
    disg_np = np.zeros((C, 128, T), np.float32)
    dise_np = np.zeros((C, 128, T), np.float32)
    disi_np = np.zeros((C, 128, T), np.float32)
    for c in range(C):
        xs = x[c * NLOC:(c + 1) * NLOC]
        xT[c, :, :NLOC] = xs.T.astype(f8)
        dl_ = np.pad(dis[c * NLOC:(c + 1) * NLOC], (0, NPAD - NLOC))
        di_ = np.pad(1.0 / dis[c * NLOC:(c + 1) * NLOC], (0, NPAD - NLOC))
        dise_np[c] = dl_.reshape(T, 128).T
        disg_np[c] = dise_np[c] / WSCALE
        disi_np[c] = di_.reshape(T, 128).T

    bias_full = np.tile(np.asarray(bias, np.float32)[None, :], (128, 1))

    return dict(
        NB=NB, idx=idx_np, oh=oh_np, w=w_f8, xT=xT,
        disg=disg_np, dise=dise_np, disi=disi_np,
        bias=np.ascontiguousarray(bias_full),
    )


def _build(NB, idx_cols, blk_cols):
    nc = bacc.Bacc("TRN2", target_bir_lowering=False, debug=False,
                   num_devices=C, num_swdge_queues=NQ)

    xT_t = nc.dram_tensor("xT", [D_IN, NPAD], FP8, kind="ExternalInput")
    w_t = nc.dram_tensor("w", [D_IN, D_OUT], FP8, kind="ExternalInput")
    disg_t = nc.dram_tensor("disg", [128, T], F32, kind="ExternalInput")
    dise_t = nc.dram_tensor("dise", [128, T], F32, kind="ExternalInput")
    disi_t = nc.dram_tensor("disi", [128, T], F32, kind="ExternalInput")
    bias_t = nc.dram_tensor("biasf", [128, D_OUT], F32, kind="ExternalInput")
    idx_t = nc.dram_tensor("idx", [128, idx_cols], I16, kind="ExternalInput")
    oh_t = nc.dram_tensor("oh", [128, blk_cols * 128], FP8,
                          kind="ExternalInput")
    out_t = nc.dram_tensor("out", [NPAD, D_OUT], F32, kind="ExternalOutput")

    xT, w, disg, dise, disi, biasf, idx, oh, out = (
        t.ap() for t in (xT_t, w_t, disg_t, dise_t, disi_t, bias_t, idx_t,
                         oh_t, out_t))

    BMAX = int(NB.max())

    # per-(tile, space) column offsets into idx / oh
    icol0 = np.zeros((T, 2), np.int64)
    bcol0 = np.zeros((T, 2), np.int64)
    ic = bc = 0
    for t in range(T):
        for h in range(2):
            icol0[t, h] = ic
            ic += 8 * int(NB[t, h])
            bcol0[t, h] = bc
            bc += int(NB[t, h])

    with tile.TileContext(nc) as tc:
        with tc.tile_pool(name="const", bufs=1) as constp, \
             tc.tile_pool(name="xk", bufs=4) as xkp, \
             tc.tile_pool(name="gath", bufs=DEPTH) as gp, \
             tc.tile_pool(name="ohp", bufs=4) as ohp, \
             tc.tile_pool(name="epi", bufs=4) as epip, \
             tc.tile_pool(name="zc", bufs=1) as zcp, \
             tc.tile_pool(name="psum", bufs=4, space="PSUM") as psp, \
             tc.tile_pool(name="dram", bufs=1, space="DRAM") as dramp:

            # resident constants
            w_sb = constp.tile([128, KT, D_OUT], FP8)
            for k in range(KT):
                nc.sync.dma_start(out=w_sb[:, k, :], in_=w[k * 128:(k + 1) * 128, :])
            disg_sb = constp.tile([128, T], F32)
            nc.sync.dma_start(out=disg_sb[:], in_=disg[:])
            dise_sb = constp.tile([128, T], F32)
            nc.sync.dma_start(out=dise_sb[:], in_=dise[:])
            disi_sb = constp.tile([128, T], F32)
            nc.sync.dma_start(out=disi_sb[:], in_=disi[:])
            bias_sb = constp.tile([128, D_OUT], F32)
            nc.sync.dma_start(out=bias_sb[:], in_=biasf[:])
            idx_sb = constp.tile([128, idx_cols], I16)
            nc.sync.dma_start(out=idx_sb[:], in_=idx[:])

            hres = constp.tile([128, T, D_OUT], FP8)     # h' kept on-chip
            partial = constp.tile([128, T, D_OUT], FP8)  # running partials
            sm = constp.tile([128, T], F32)              # exp row-sums
            nm2s = constp.tile([128, T], F32)            # -dis*rowmax stash

            h_loc = [dramp.tile([LOCS[h], D_OUT], FP8, name=f"h_loc{h}")
                     for h in range(2)]
            h_all = [dramp.tile([C * LOCS[h], D_OUT], FP8,
                                addr_space="Shared", name=f"h_all{h}")
                     for h in range(2)]

            # ---- phase 1: h' = dis * (x @ W) in fp8, two allgathers ----
            for s in range(NCHUNK):
                sw = SW if s < NCHUNK - 1 else NPAD - (NCHUNK - 1) * SW
                xk = xkp.tile([128, KT, SW], FP8, name="xk", tag="xk")
                for k in range(KT):
                    nc.sync.dma_start(
                        out=xk[:, k, :sw],
                        in_=xT[k * 128:(k + 1) * 128, s * SW:s * SW + sw])
                for t in range(sw // 128):
                    ph = psp.tile([128, D_OUT], F32, name="ph", tag="ph")
                    for k in range(KT):
                        nc.tensor.matmul(
                            ph[:], xk[:, k, t * 128:(t + 1) * 128],
                            w_sb[:, k, :], start=(k == 0), stop=(k == KT - 1))
                    gt = s * (SW // 128) + t
                    # h' = (x@64W) * dis/64 on the Scalar engine
                    nc.scalar.activation(hres[:, gt, :], ph[:],
                                         mybir.ActivationFunctionType.Copy,
                                         scale=disg_sb[:, gt:gt + 1])
                    # pre-fold bias/dis + self term into the partial buffer
                    nc.vector.tensor_scalar(partial[:, gt, :], bias_sb[:],
                                            disi_sb[:, gt:gt + 1], None,
                                            mybir.AluOpType.mult)
                    nc.vector.tensor_tensor(partial[:, gt, :],
                                            partial[:, gt, :], hres[:, gt, :],
                                            mybir.AluOpType.add)
                    r0 = gt * 128
                    h = 0 if gt < TILE_A else 1
                    ro = r0 - EDGE0[h]
                    nc.sync.dma_start(out=h_loc[h][ro:ro + 128, :],
                                      in_=hres[:, gt, :])
                    if gt == TILE_A - 1:
                        nc.gpsimd.collective_compute(
                            "AllGather", mybir.AluOpType.bypass,
                            replica_groups=[list(range(C))],
                            ins=[h_loc[0].opt()], outs=[h_all[0].opt()])
            nc.gpsimd.collective_compute(
                "AllGather", mybir.AluOpType.bypass,
                replica_groups=[list(range(C))],
                ins=[h_loc[1].opt()], outs=[h_all[1].opt()])

            qc = 0  # gather queue round-robin counter

            def do_space(t, h, tag):
                """Gather + one-hot matmuls for (tile t, space h) -> psum."""
                nonlocal qc
                nblk = int(NB[t, h])
                ga = gp.tile([128, BMAX, D_OUT], FP8, name="g" + tag, tag="g")
                nc.gpsimd.dma_gather(
                    out_ap=ga[:, :nblk, :], in_ap=h_all[h][:],
                    idxs_ap=idx_sb[:, int(icol0[t, h]):int(icol0[t, h]) + 8 * nblk],
                    num_idxs=nblk * 128, num_idxs_reg=nblk * 128,
                    elem_size=D_OUT, queue_num=qc % NQ)
                qc += 1
                ohs = ohp.tile([128, BMAX * 128], FP8, name="oh" + tag,
                               tag="oh")
                b0 = int(bcol0[t, h])
                nc.sync.dma_start(out=ohs[:, :nblk * 128],
                                  in_=oh[:, b0 * 128:(b0 + nblk) * 128])
                acc = psp.tile([128, D_OUT], F32, name="p" + tag, tag="p")
                for b in range(nblk):
                    nc.tensor.matmul(
                        acc[:], ohs[:, b * 128:(b + 1) * 128],
                        ga[:, b, :], start=(b == 0), stop=(b == nblk - 1))
                return acc

            # ---- phase 3a: space-0 gathers, accumulate into partials ----
            for t in range(T):
                if NB[t, 0] == 0:
                    continue
                pa = do_space(t, 0, "a")
                nc.vector.tensor_tensor(partial[:, t, :], pa[:],
                                        partial[:, t, :], mybir.AluOpType.add)

            # ---- phase 3c: space-1 gathers + epilogue ----
            # z_scaled = dis*(acc+partial); Exp(scale*z + bias) does the
            # dis-mult and max-shift inside the Scalar activation.
            for t in range(T):
                nb2 = int(NB[t, 1])
                if t % EG == 0:
                    zc = zcp.tile([128, EG, D_OUT], F32, name="zc", tag="zc")
                pos = t % EG
                if nb2:
                    acc = do_space(t, 1, "c")
                    nc.vector.tensor_tensor(zc[:, pos, :], acc[:],
                                            partial[:, t, :],
                                            mybir.AluOpType.add)
                else:
                    nc.vector.tensor_copy(zc[:, pos, :], partial[:, t, :])
                mx = epip.tile([128, 1], F32, name="mx", tag="mx")
                nc.vector.tensor_reduce(mx[:], zc[:, pos, :],
                                        mybir.AxisListType.X,
                                        mybir.AluOpType.max)
                # nm2s = -dis*rowmax  (the Exp shift)
                nc.vector.tensor_scalar(nm2s[:, t:t + 1], mx[:],
                                        dise_sb[:, t:t + 1], -1.0,
                                        mybir.AluOpType.mult,
                                        mybir.AluOpType.mult)
                ex = epip.tile([128, D_OUT], FP8, name="ex", tag="ex")
                nc.scalar.activation(ex[:], zc[:, pos, :],
                                     mybir.ActivationFunctionType.Exp,
                                     bias=nm2s[:, t:t + 1],
                                     scale=dise_sb[:, t:t + 1],
                                     accum_out=sm[:, t:t + 1])
                if pos == EG - 1:
                    g0 = t - (EG - 1)
                    lse = epip.tile([128, EG], F32, name="lse", tag="lse")
                    nc.scalar.activation(lse[:], sm[:, g0:g0 + EG],
                                         mybir.ActivationFunctionType.Ln)
                    qg = epip.tile([128, EG], F32, name="qg", tag="qg")
                    nc.vector.tensor_tensor(qg[:], nm2s[:, g0:g0 + EG], lse[:],
                                            mybir.AluOpType.subtract)
                    for p in range(EG):
                        res = epip.tile([128, D_OUT], F32, name="res",
                                        tag="res")
                        nc.vector.tensor_scalar(res[:], zc[:, p, :],
                                                dise_sb[:, g0 + p:g0 + p + 1],
                                                qg[:, p:p + 1],
                                                mybir.AluOpType.mult,
                                                mybir.AluOpType.add)
                        nc.sync.dma_start(
                            out=out[(g0 + p) * 128:(g0 + p + 1) * 128, :],
                            in_=res[:])

    nc.compile()
    return nc


def kernel(x, edge_index, weight, bias):
    global LAST_RESULTS
    x = np.asarray(x, dtype=np.float32)
    weight = np.asarray(weight, dtype=np.float32)
    bias = np.asarray(bias, dtype=np.float32)

    pp = _preprocess(x, edge_index, weight, bias)
    idx_cols = pp["idx"].shape[2]
    blk_cols = pp["oh"].shape[2] // 128
    nc = _build(pp["NB"], idx_cols, blk_cols)

    in_maps = []
    for c in range(C):
        in_maps.append({
            "xT": np.ascontiguousarray(pp["xT"][c]),
            "w": pp["w"],
            "disg": np.ascontiguousarray(pp["disg"][c]),
            "dise": np.ascontiguousarray(pp["dise"][c]),
            "disi": np.ascontiguousarray(pp["disi"][c]),
            "biasf": pp["bias"],
            "idx": np.ascontiguousarray(pp["idx"][c]),
            "oh": np.ascontiguousarray(pp["oh"][c]),
        })

    res = run_bass_kernel_spmd(nc, in_maps, core_ids=list(range(C)))
    LAST_RESULTS = res

    out = np.empty((N_NODES, D_OUT), np.float32)
    for c in range(C):
        out[c * NLOC:(c + 1) * NLOC] = res.results[c]["out"][:NLOC]
    return out


# revision 10
# speedup vs baseline: 1.0397x; 1.0397x over previous
"""GCN layer (GCNConv + log_softmax) on 8 Trainium2 NeuronCores.

Sharding: nodes row-sharded 8 ways. Each core computes h' = dis * (x @ W)
for its slice (fp8-e3m4 GEMM with W pre-scaled by 64, rescaled in the
Scalar-engine copy), AllGathers h' in two chunks (A: rows 0-3967 fired
mid-GEMM, B: the 2304-row tail fired at GEMM end) and aggregates
messages for its destination slice with dma_gather calls spread over 4
SWDGE queues (parallel Q7 descriptor generation) + host-built fp8
one-hot indicator matmuls on the tensor engine.  Self-loop and bias
terms are pre-folded into the per-tile partial buffer during the GEMM,
so the epilogue is only: z = acc + partial, row-max, then log_softmax
with the dis/shift folded into the Scalar activation's scale/bias
operands.
"""

import numpy as np
import ml_dtypes

import concourse.bass as bass
import concourse.tile as tile
from concourse import bacc, mybir
from concourse.bass_utils import run_bass_kernel_spmd

bf16 = ml_dtypes.bfloat16
f8 = ml_dtypes.float8_e3m4
F32 = mybir.dt.float32
BF16 = mybir.dt.bfloat16
FP8 = mybir.dt.float8e3
I16 = mybir.dt.int16

N_NODES = 50000
D_IN = 2048
D_OUT = 512
C = 8                      # cores
NLOC = N_NODES // C        # 6250 real nodes per core
T = 49                     # dst tiles per core
NPAD = T * 128             # 6272 padded rows per core
SW = 512                   # GEMM node-chunk width
NCHUNK = 13                # 12 full chunks + 128-row tail
WSCALE = 64.0              # weight pre-scale so fp8-e3m4 w stays normal
TILE_A = 24                # dst tiles in gather space A
LOCS = (TILE_A * 128, NPAD - TILE_A * 128)     # (3968, 2304) rows per core
EDGE0 = (0, TILE_A * 128)
KT = D_IN // 128           # 16 contraction chunks
DEPTH = 12                 # gather tile pool depth
NQ = 4                     # SWDGE queues (parallel Q7 descriptor gen)
EG = 7                     # epilogue group size (tiles per Ln batch)

LAST_RESULTS = None        # test harness reads exec_time_ns from here


def _wrap_idx(idx):
    """Wrap a [n] index array into the [128, n//16] dma_gather layout."""
    n = idx.shape[0]
    assert n % 16 == 0
    cols = n // 16
    w = np.empty((128, cols), np.int16)
    blk = idx.reshape(cols, 16).T.astype(np.int16)   # [16, cols]
    for g in range(8):
        w[g * 16:(g + 1) * 16, :] = blk
    return w


def _preprocess(x, edge_index, weight, bias):
    src = np.asarray(edge_index[0], dtype=np.int64)
    dst = np.asarray(edge_index[1], dtype=np.int64)

    # degree includes the self loop; self-loop messages are served from the
    # SBUF copy of h' instead of being gathered.
    deg = (np.bincount(dst, minlength=N_NODES) + 1).astype(np.float32)
    dis = 1.0 / np.sqrt(deg)

    sc = src // NLOC
    sr = src % NLOC
    half = (sr >= EDGE0[1]).astype(np.int64)         # gather space 0/1
    base = np.asarray(LOCS)[half]
    g = sc * base + (sr - np.asarray(EDGE0)[half])

    dc = dst // NLOC                   # dst core
    dr = dst % NLOC
    dt = dr // 128                     # dst tile within core
    dl = dr % 128                      # dst row within tile

    order = np.lexsort((g, half, dt, dc))
    g, dc, dt, dl, half = g[order], dc[order], dt[order], dl[order], half[order]

    key = (dc * T + dt) * 2 + half
    counts = np.bincount(key, minlength=C * T * 2).reshape(C, T, 2)
    blocks = -(-counts // 128)                       # ceil div
    NB = blocks.max(axis=0)                          # [T, 2]

    blk_cols = int(NB.sum())
    idx_cols = 8 * blk_cols
    idx_np = np.zeros((C, 128, idx_cols), np.int16)
    oh_np = np.zeros((C, 128, blk_cols * 128), f8)
    dcol = np.arange(128)

    starts = np.zeros(C * T * 2 + 1, np.int64)
    np.cumsum(np.bincount(key, minlength=C * T * 2), out=starts[1:])

    for c in range(C):
        icol = 0
        bcol = 0
        for t in range(T):
            for h in range(2):
                B = int(NB[t, h])
                if B == 0:
                    continue
                k = (c * T + t) * 2 + h
                seg = slice(starts[k], starts[k + 1])
                n = starts[k + 1] - starts[k]
                cap = B * 128
                gi = np.zeros(cap, np.int64)
                gi[:n] = g[seg]
                dv = np.full(cap, -1.0, np.float32)
                dv[:n] = dl[seg]
                idx_np[c, :, icol:icol + 8 * B] = _wrap_idx(gi)
                ohb = (dv.reshape(B, 128)[:, :, None] == dcol[None, None, :])
                oh_np[c, :, bcol * 128:(bcol + B) * 128] = (
                    ohb.transpose(1, 0, 2).reshape(128, B * 128).astype(f8))
                icol += 8 * B
                bcol += B

    w_f8 = np.ascontiguousarray((weight * WSCALE).astype(f8))
    xT = np.zeros((C, D_IN, NPAD), f8)
    disg_np = np.zeros((C, 128, T), np.float32)
    dise_np = np.zeros((C, 128, T), np.float32)
    disi_np = np.zeros((C, 128, T), np.float32)
    for c in range(C):
        xs = x[c * NLOC:(c + 1) * NLOC]
        xT[c, :, :NLOC] = xs.T.astype(f8)
        dl_ = np.pad(dis[c * NLOC:(c + 1) * NLOC], (0, NPAD - NLOC))
        di_ = np.pad(1.0 / dis[c * NLOC:(c + 1) * NLOC], (0, NPAD - NLOC))
        dise_np[c] = dl_.reshape(T, 128).T
        disg_np[c] = dise_np[c] / WSCALE
        disi_np[c] = di_.reshape(T, 128).T

    bias_full = np.tile(np.asarray(bias, np.float32)[None, :], (128, 1))

    return dict(
        NB=NB, idx=idx_np, oh=oh_np, w=w_f8, xT=xT,
        disg=disg_np, dise=dise_np, disi=disi_np,
        bias=np.ascontiguousarray(bias_full),
    )


def _build(NB, idx_cols, blk_cols):
    nc = bacc.Bacc("TRN2", target_bir_lowering=False, debug=False,
                   num_devices=C, num_swdge_queues=NQ)

    xT_t = nc.dram_tensor("xT", [D_IN, NPAD], FP8, kind="ExternalInput")
    w_t = nc.dram_tensor("w", [D_IN, D_OUT], FP8, kind="ExternalInput")
    disg_t = nc.dram_tensor("disg", [128, T], F32, kind="ExternalInput")
    dise_t = nc.dram_tensor("dise", [128, T], F32, kind="ExternalInput")
    disi_t = nc.dram_tensor("disi", [128, T], F32, kind="ExternalInput")
    bias_t = nc.dram_tensor("biasf", [128, D_OUT], F32, kind="ExternalInput")
    idx_t = nc.dram_tensor("idx", [128, idx_cols], I16, kind="ExternalInput")
    oh_t = nc.dram_tensor("oh", [128, blk_cols * 128], FP8,
                          kind="ExternalInput")
    out_t = nc.dram_tensor("out", [NPAD, D_OUT], F32, kind="ExternalOutput")

    xT, w, disg, dise, disi, biasf, idx, oh, out = (
        t.ap() for t in (xT_t, w_t, disg_t, dise_t, disi_t, bias_t, idx_t,
                         oh_t, out_t))

    BMAX = int(NB.max())

    # per-(tile, space) column offsets into idx / oh
    icol0 = np.zeros((T, 2), np.int64)
    bcol0 = np.zeros((T, 2), np.int64)
    ic = bc = 0
    for t in range(T):
        for h in range(2):
            icol0[t, h] = ic
            ic += 8 * int(NB[t, h])
            bcol0[t, h] = bc
            bc += int(NB[t, h])

    with tile.TileContext(nc) as tc:
        with tc.tile_pool(name="const", bufs=1) as constp, \
             tc.tile_pool(name="xk", bufs=4) as xkp, \
             tc.tile_pool(name="gath", bufs=DEPTH) as gp, \
             tc.tile_pool(name="ohp", bufs=4) as ohp, \
             tc.tile_pool(name="epi", bufs=4) as epip, \
             tc.tile_pool(name="zc", bufs=2) as zcp, \
             tc.tile_pool(name="psum", bufs=4, space="PSUM") as psp, \
             tc.tile_pool(name="dram", bufs=1, space="DRAM") as dramp:

            # resident constants
            w_sb = constp.tile([128, KT, D_OUT], FP8)
            for k in range(KT):
                nc.sync.dma_start(out=w_sb[:, k, :], in_=w[k * 128:(k + 1) * 128, :])
            disg_sb = constp.tile([128, T], F32)
            nc.sync.dma_start(out=disg_sb[:], in_=disg[:])
            dise_sb = constp.tile([128, T], F32)
            nc.sync.dma_start(out=dise_sb[:], in_=dise[:])
            disi_sb = constp.tile([128, T], F32)
            nc.sync.dma_start(out=disi_sb[:], in_=disi[:])
            bias_sb = constp.tile([128, D_OUT], F32)
            nc.sync.dma_start(out=bias_sb[:], in_=biasf[:])
            idx_sb = constp.tile([128, idx_cols], I16)
            nc.sync.dma_start(out=idx_sb[:], in_=idx[:])

            hres = constp.tile([128, T, D_OUT], FP8)     # h' kept on-chip
            partial = constp.tile([128, T, D_OUT], FP8)  # running partials
            sm = constp.tile([128, T], F32)              # exp row-sums
            nm2s = constp.tile([128, T], F32)            # -dis*rowmax stash

            h_loc = [dramp.tile([LOCS[h], D_OUT], FP8, name=f"h_loc{h}")
                     for h in range(2)]
            h_all = [dramp.tile([C * LOCS[h], D_OUT], FP8,
                                addr_space="Shared", name=f"h_all{h}")
                     for h in range(2)]

            # ---- phase 1: h' = dis * (x @ W) in fp8, two allgathers ----
            for s in range(NCHUNK):
                sw = SW if s < NCHUNK - 1 else NPAD - (NCHUNK - 1) * SW
                xk = xkp.tile([128, KT, SW], FP8, name="xk", tag="xk")
                for k in range(KT):
                    nc.sync.dma_start(
                        out=xk[:, k, :sw],
                        in_=xT[k * 128:(k + 1) * 128, s * SW:s * SW + sw])
                for t in range(sw // 128):
                    ph = psp.tile([128, D_OUT], F32, name="ph", tag="ph")
                    for k in range(KT):
                        nc.tensor.matmul(
                            ph[:], xk[:, k, t * 128:(t + 1) * 128],
                            w_sb[:, k, :], start=(k == 0), stop=(k == KT - 1))
                    gt = s * (SW // 128) + t
                    # h' = (x@64W) * dis/64 on the Scalar engine
                    nc.scalar.activation(hres[:, gt, :], ph[:],
                                         mybir.ActivationFunctionType.Copy,
                                         scale=disg_sb[:, gt:gt + 1])
                    # pre-fold bias/dis + self term into the partial buffer
                    nc.vector.tensor_scalar(partial[:, gt, :], bias_sb[:],
                                            disi_sb[:, gt:gt + 1], None,
                                            mybir.AluOpType.mult)
                    nc.vector.tensor_tensor(partial[:, gt, :],
                                            partial[:, gt, :], hres[:, gt, :],
                                            mybir.AluOpType.add)
                    r0 = gt * 128
                    h = 0 if gt < TILE_A else 1
                    ro = r0 - EDGE0[h]
                    nc.sync.dma_start(out=h_loc[h][ro:ro + 128, :],
                                      in_=hres[:, gt, :])
                    if gt == TILE_A - 1:
                        nc.gpsimd.collective_compute(
                            "AllGather", mybir.AluOpType.bypass,
                            replica_groups=[list(range(C))],
                            ins=[h_loc[0].opt()], outs=[h_all[0].opt()])
            nc.gpsimd.collective_compute(
                "AllGather", mybir.AluOpType.bypass,
                replica_groups=[list(range(C))],
                ins=[h_loc[1].opt()], outs=[h_all[1].opt()])

            qc = 0  # gather queue round-robin counter

            def do_space(t, h, tag):
                """Gather + one-hot matmuls for (tile t, space h) -> psum."""
                nonlocal qc
                nblk = int(NB[t, h])
                ga = gp.tile([128, BMAX, D_OUT], FP8, name="g" + tag, tag="g")
                nc.gpsimd.dma_gather(
                    out_ap=ga[:, :nblk, :], in_ap=h_all[h][:],
                    idxs_ap=idx_sb[:, int(icol0[t, h]):int(icol0[t, h]) + 8 * nblk],
                    num_idxs=nblk * 128, num_idxs_reg=nblk * 128,
                    elem_size=D_OUT, queue_num=qc % NQ)
                qc += 1
                ohs = ohp.tile([128, BMAX * 128], FP8, name="oh" + tag,
                               tag="oh")
                b0 = int(bcol0[t, h])
                nc.sync.dma_start(out=ohs[:, :nblk * 128],
                                  in_=oh[:, b0 * 128:(b0 + nblk) * 128])
                acc = psp.tile([128, D_OUT], F32, name="p" + tag, tag="p")
                for b in range(nblk):
                    nc.tensor.matmul(
                        acc[:], ohs[:, b * 128:(b + 1) * 128],
                        ga[:, b, :], start=(b == 0), stop=(b == nblk - 1))
                return acc

            # ---- phase 3a: space-0 gathers, accumulate into partials ----
            for t in range(T):
                if NB[t, 0] == 0:
                    continue
                pa = do_space(t, 0, "a")
                nc.vector.tensor_tensor(partial[:, t, :], pa[:],
                                        partial[:, t, :], mybir.AluOpType.add)

            # ---- phase 3c: space-1 gathers + epilogue ----
            # z_scaled = dis*(acc+partial); Exp(scale*z + bias) does the
            # dis-mult and max-shift inside the Scalar activation.
            for t in range(T):
                nb2 = int(NB[t, 1])
                if t % EG == 0:
                    zc = zcp.tile([128, EG, D_OUT], F32, name="zc", tag="zc")
                pos = t % EG
                if nb2:
                    acc = do_space(t, 1, "c")
                    nc.vector.tensor_tensor(zc[:, pos, :], acc[:],
                                            partial[:, t, :],
                                            mybir.AluOpType.add)
                else:
                    nc.vector.tensor_copy(zc[:, pos, :], partial[:, t, :])
                mx = epip.tile([128, 1], F32, name="mx", tag="mx")
                nc.vector.tensor_reduce(mx[:], zc[:, pos, :],
                                        mybir.AxisListType.X,
                                        mybir.AluOpType.max)
                # nm2s = -dis*rowmax  (the Exp shift)
                nc.vector.tensor_scalar(nm2s[:, t:t + 1], mx[:],
                                        dise_sb[:, t:t + 1], -1.0,
                                        mybir.AluOpType.mult,
                                        mybir.AluOpType.mult)
                ex = epip.tile([128, D_OUT], FP8, name="ex", tag="ex")
                nc.scalar.activation(ex[:], zc[:, pos, :],
                                     mybir.ActivationFunctionType.Exp,
                                     bias=nm2s[:, t:t + 1],
                                     scale=dise_sb[:, t:t + 1],
                                     accum_out=sm[:, t:t + 1])
                if pos == EG - 1:
                    g0 = t - (EG - 1)
                    lse = epip.tile([128, EG], F32, name="lse", tag="lse")
                    nc.scalar.activation(lse[:], sm[:, g0:g0 + EG],
                                         mybir.ActivationFunctionType.Ln)
                    qg = epip.tile([128, EG], F32, name="qg", tag="qg")
                    nc.vector.tensor_tensor(qg[:], nm2s[:, g0:g0 + EG], lse[:],
                                            mybir.AluOpType.subtract)
                    for p in range(EG):
                        res = epip.tile([128, D_OUT], F32, name="res",
                                        tag="res")
                        nc.vector.tensor_scalar(res[:], zc[:, p, :],
                                                dise_sb[:, g0 + p:g0 + p + 1],
                                                qg[:, p:p + 1],
                                                mybir.AluOpType.mult,
                                                mybir.AluOpType.add)
                        nc.sync.dma_start(
                            out=out[(g0 + p) * 128:(g0 + p + 1) * 128, :],
                            in_=res[:])

    nc.compile()
    return nc


def kernel(x, edge_index, weight, bias):
    global LAST_RESULTS
    x = np.asarray(x, dtype=np.float32)
    weight = np.asarray(weight, dtype=np.float32)
    bias = np.asarray(bias, dtype=np.float32)

    pp = _preprocess(x, edge_index, weight, bias)
    idx_cols = pp["idx"].shape[2]
    blk_cols = pp["oh"].shape[2] // 128
    nc = _build(pp["NB"], idx_cols, blk_cols)

    in_maps = []
    for c in range(C):
        in_maps.append({
            "xT": np.ascontiguousarray(pp["xT"][c]),
            "w": pp["w"],
            "disg": np.ascontiguousarray(pp["disg"][c]),
            "dise": np.ascontiguousarray(pp["dise"][c]),
            "disi": np.ascontiguousarray(pp["disi"][c]),
            "biasf": pp["bias"],
            "idx": np.ascontiguousarray(pp["idx"][c]),
            "oh": np.ascontiguousarray(pp["oh"][c]),
        })

    res = run_bass_kernel_spmd(nc, in_maps, core_ids=list(range(C)))
    LAST_RESULTS = res

    out = np.empty((N_NODES, D_OUT), np.float32)
    for c in range(C):
        out[c * NLOC:(c + 1) * NLOC] = res.results[c]["out"][:NLOC]
    return out


# revision 12
# speedup vs baseline: 1.0713x; 1.0304x over previous
"""GCN layer (GCNConv + log_softmax) on 8 Trainium2 NeuronCores.

Sharding: nodes row-sharded 8 ways. Each core computes h' = dis * (x @ W)
for its slice (fp8-e3m4 GEMM with W pre-scaled by 64, rescaled in the
Scalar-engine copy), AllGathers h' in two chunks (A: rows 0-3967 fired
mid-GEMM, B: the 2304-row tail fired at GEMM end) and aggregates
messages for its destination slice with dma_gather calls spread over 4
SWDGE queues (parallel Q7 descriptor generation) + host-built fp8
one-hot indicator matmuls on the tensor engine.  Self-loop and bias
terms are pre-folded into the per-tile partial buffer during the GEMM,
so the epilogue is only: z = acc + partial, row-max, then log_softmax
with the dis/shift folded into the Scalar activation's scale/bias
operands.
"""

import numpy as np
import ml_dtypes

import concourse.bass as bass
import concourse.tile as tile
from concourse import bacc, mybir
from concourse.bass import _add_dep_helper
from concourse.bass_utils import run_bass_kernel_spmd

bf16 = ml_dtypes.bfloat16
f8 = ml_dtypes.float8_e3m4
F32 = mybir.dt.float32
BF16 = mybir.dt.bfloat16
FP8 = mybir.dt.float8e3
I16 = mybir.dt.int16

N_NODES = 50000
D_IN = 2048
D_OUT = 512
C = 8                      # cores
NLOC = N_NODES // C        # 6250 real nodes per core
T = 49                     # dst tiles per core
NPAD = T * 128             # 6272 padded rows per core
SW = 512                   # GEMM node-chunk width
NCHUNK = 13                # 12 full chunks + 128-row tail
WSCALE = 64.0              # weight pre-scale so fp8-e3m4 w stays normal
TILE_A = 24                # dst tiles in gather space A
LOCS = (TILE_A * 128, NPAD - TILE_A * 128)     # (3968, 2304) rows per core
EDGE0 = (0, TILE_A * 128)
KT = D_IN // 128           # 16 contraction chunks
DEPTH = 12                 # gather tile pool depth
NQ = 4                     # SWDGE queues (parallel Q7 descriptor gen)
EG = 7                     # epilogue group size (tiles per Ln batch)

LAST_RESULTS = None        # test harness reads exec_time_ns from here


def _wrap_idx(idx):
    """Wrap a [n] index array into the [128, n//16] dma_gather layout."""
    n = idx.shape[0]
    assert n % 16 == 0
    cols = n // 16
    w = np.empty((128, cols), np.int16)
    blk = idx.reshape(cols, 16).T.astype(np.int16)   # [16, cols]
    for g in range(8):
        w[g * 16:(g + 1) * 16, :] = blk
    return w


def _preprocess(x, edge_index, weight, bias):
    src = np.asarray(edge_index[0], dtype=np.int64)
    dst = np.asarray(edge_index[1], dtype=np.int64)

    # degree includes the self loop; self-loop messages are served from the
    # SBUF copy of h' instead of being gathered.
    deg = (np.bincount(dst, minlength=N_NODES) + 1).astype(np.float32)
    dis = 1.0 / np.sqrt(deg)

    sc = src // NLOC
    sr = src % NLOC
    half = (sr >= EDGE0[1]).astype(np.int64)         # gather space 0/1
    base = np.asarray(LOCS)[half]
    g = sc * base + (sr - np.asarray(EDGE0)[half])

    dc = dst // NLOC                   # dst core
    dr = dst % NLOC
    dt = dr // 128                     # dst tile within core
    dl = dr % 128                      # dst row within tile

    order = np.lexsort((g, half, dt, dc))
    g, dc, dt, dl, half = g[order], dc[order], dt[order], dl[order], half[order]

    key = (dc * T + dt) * 2 + half
    counts = np.bincount(key, minlength=C * T * 2).reshape(C, T, 2)
    blocks = -(-counts // 128)                       # ceil div
    NB = blocks.max(axis=0)                          # [T, 2]

    blk_cols = int(NB.sum())
    idx_cols = 8 * blk_cols
    idx_np = np.zeros((C, 128, idx_cols), np.int16)
    oh_np = np.zeros((C, 128, blk_cols * 128), f8)
    dcol = np.arange(128)

    starts = np.zeros(C * T * 2 + 1, np.int64)
    np.cumsum(np.bincount(key, minlength=C * T * 2), out=starts[1:])

    for c in range(C):
        icol = 0
        bcol = 0
        for t in range(T):
            for h in range(2):
                B = int(NB[t, h])
                if B == 0:
                    continue
                k = (c * T + t) * 2 + h
                seg = slice(starts[k], starts[k + 1])
                n = starts[k + 1] - starts[k]
                cap = B * 128
                gi = np.zeros(cap, np.int64)
                gi[:n] = g[seg]
                dv = np.full(cap, -1.0, np.float32)
                dv[:n] = dl[seg]
                idx_np[c, :, icol:icol + 8 * B] = _wrap_idx(gi)
                ohb = (dv.reshape(B, 128)[:, :, None] == dcol[None, None, :])
                oh_np[c, :, bcol * 128:(bcol + B) * 128] = (
                    ohb.transpose(1, 0, 2).reshape(128, B * 128).astype(f8))
                icol += 8 * B
                bcol += B

    w_f8 = np.ascontiguousarray((weight * WSCALE).astype(f8))
    xT = np.zeros((C, D_IN, NPAD), f8)
    disg_np = np.zeros((C, 128, T), np.float32)
    dise_np = np.zeros((C, 128, T), np.float32)
    disi_np = np.zeros((C, 128, T), np.float32)
    for c in range(C):
        xs = x[c * NLOC:(c + 1) * NLOC]
        xT[c, :, :NLOC] = xs.T.astype(f8)
        dl_ = np.pad(dis[c * NLOC:(c + 1) * NLOC], (0, NPAD - NLOC))
        di_ = np.pad(1.0 / dis[c * NLOC:(c + 1) * NLOC], (0, NPAD - NLOC))
        dise_np[c] = dl_.reshape(T, 128).T
        disg_np[c] = dise_np[c] / WSCALE
        disi_np[c] = di_.reshape(T, 128).T

    bias_full = np.tile(np.asarray(bias, np.float32)[None, :], (128, 1))

    return dict(
        NB=NB, idx=idx_np, oh=oh_np, w=w_f8, xT=xT,
        disg=disg_np, dise=dise_np, disi=disi_np,
        bias=np.ascontiguousarray(bias_full),
    )


def _build(NB, idx_cols, blk_cols):
    nc = bacc.Bacc("TRN2", target_bir_lowering=False, debug=False,
                   num_devices=C, num_swdge_queues=NQ)

    xT_t = nc.dram_tensor("xT", [D_IN, NPAD], FP8, kind="ExternalInput")
    w_t = nc.dram_tensor("w", [D_IN, D_OUT], FP8, kind="ExternalInput")
    disg_t = nc.dram_tensor("disg", [128, T], F32, kind="ExternalInput")
    dise_t = nc.dram_tensor("dise", [128, T], F32, kind="ExternalInput")
    disi_t = nc.dram_tensor("disi", [128, T], F32, kind="ExternalInput")
    bias_t = nc.dram_tensor("biasf", [128, D_OUT], F32, kind="ExternalInput")
    idx_t = nc.dram_tensor("idx", [128, idx_cols], I16, kind="ExternalInput")
    oh_t = nc.dram_tensor("oh", [128, blk_cols * 128], FP8,
                          kind="ExternalInput")
    out_t = nc.dram_tensor("out", [NPAD, D_OUT], F32, kind="ExternalOutput")

    xT, w, disg, dise, disi, biasf, idx, oh, out = (
        t.ap() for t in (xT_t, w_t, disg_t, dise_t, disi_t, bias_t, idx_t,
                         oh_t, out_t))

    BMAX = int(NB.max())

    # per-(tile, space) column offsets into idx / oh
    icol0 = np.zeros((T, 2), np.int64)
    bcol0 = np.zeros((T, 2), np.int64)
    ic = bc = 0
    for t in range(T):
        for h in range(2):
            icol0[t, h] = ic
            ic += 8 * int(NB[t, h])
            bcol0[t, h] = bc
            bc += int(NB[t, h])

    with tile.TileContext(nc) as tc:
        with tc.tile_pool(name="const", bufs=1) as constp, \
             tc.tile_pool(name="xk", bufs=4) as xkp, \
             tc.tile_pool(name="gath", bufs=DEPTH) as gp, \
             tc.tile_pool(name="ohp", bufs=4) as ohp, \
             tc.tile_pool(name="epi", bufs=4) as epip, \
             tc.tile_pool(name="zc", bufs=2) as zcp, \
             tc.tile_pool(name="psum", bufs=4, space="PSUM") as psp, \
             tc.tile_pool(name="dram", bufs=1, space="DRAM") as dramp:

            # resident constants
            w_sb = constp.tile([128, KT, D_OUT], FP8)
            for k in range(KT):
                nc.sync.dma_start(out=w_sb[:, k, :], in_=w[k * 128:(k + 1) * 128, :])
            disg_sb = constp.tile([128, T], F32)
            nc.sync.dma_start(out=disg_sb[:], in_=disg[:])
            dise_sb = constp.tile([128, T], F32)
            nc.sync.dma_start(out=dise_sb[:], in_=dise[:])
            disi_sb = constp.tile([128, T], F32)
            nc.sync.dma_start(out=disi_sb[:], in_=disi[:])
            bias_sb = constp.tile([128, D_OUT], F32)
            nc.sync.dma_start(out=bias_sb[:], in_=biasf[:])
            idx_sb = constp.tile([128, idx_cols], I16)
            nc.sync.dma_start(out=idx_sb[:], in_=idx[:])

            hres = constp.tile([128, T, D_OUT], FP8)     # h' kept on-chip
            partial = constp.tile([128, T, D_OUT], FP8)  # running partials
            sm = constp.tile([128, T], F32)              # exp row-sums
            nm2s = constp.tile([128, T], F32)            # -dis*rowmax stash

            h_loc = [dramp.tile([LOCS[h], D_OUT], FP8, name=f"h_loc{h}")
                     for h in range(2)]
            h_all = [dramp.tile([C * LOCS[h], D_OUT], FP8,
                                addr_space="Shared", name=f"h_all{h}")
                     for h in range(2)]

            # ---- phase 1: h' = dis * (x @ W) in fp8, two allgathers ----
            for s in range(NCHUNK):
                sw = SW if s < NCHUNK - 1 else NPAD - (NCHUNK - 1) * SW
                xk = xkp.tile([128, KT, SW], FP8, name="xk", tag="xk")
                for k in range(KT):
                    nc.sync.dma_start(
                        out=xk[:, k, :sw],
                        in_=xT[k * 128:(k + 1) * 128, s * SW:s * SW + sw])
                for t in range(sw // 128):
                    ph = psp.tile([128, D_OUT], F32, name="ph", tag="ph")
                    for k in range(KT):
                        nc.tensor.matmul(
                            ph[:], xk[:, k, t * 128:(t + 1) * 128],
                            w_sb[:, k, :], start=(k == 0), stop=(k == KT - 1))
                    gt = s * (SW // 128) + t
                    # h' = (x@64W) * dis/64 on the Scalar engine
                    nc.scalar.activation(hres[:, gt, :], ph[:],
                                         mybir.ActivationFunctionType.Copy,
                                         scale=disg_sb[:, gt:gt + 1])
                    # pre-fold bias/dis + self term into the partial buffer
                    nc.vector.tensor_scalar(partial[:, gt, :], bias_sb[:],
                                            disi_sb[:, gt:gt + 1], None,
                                            mybir.AluOpType.mult)
                    nc.vector.tensor_tensor(partial[:, gt, :],
                                            partial[:, gt, :], hres[:, gt, :],
                                            mybir.AluOpType.add)
                    r0 = gt * 128
                    h = 0 if gt < TILE_A else 1
                    ro = r0 - EDGE0[h]
                    nc.sync.dma_start(out=h_loc[h][ro:ro + 128, :],
                                      in_=hres[:, gt, :])
                    if gt == TILE_A - 1:
                        nc.gpsimd.collective_compute(
                            "AllGather", mybir.AluOpType.bypass,
                            replica_groups=[list(range(C))],
                            ins=[h_loc[0].opt()], outs=[h_all[0].opt()])
            agb = nc.gpsimd.collective_compute(
                "AllGather", mybir.AluOpType.bypass,
                replica_groups=[list(range(C))],
                ins=[h_loc[1].opt()], outs=[h_all[1].opt()])

            qc = 0  # gather queue round-robin counter
            K_EARLY = 4   # 3a gathers allowed to run before the AG-B trigger

            def do_space(t, h, tag):
                """Gather + one-hot matmuls for (tile t, space h) -> psum."""
                nonlocal qc
                nblk = int(NB[t, h])
                ga = gp.tile([128, BMAX, D_OUT], FP8, name="g" + tag, tag="g")
                ginst = nc.gpsimd.dma_gather(
                    out_ap=ga[:, :nblk, :], in_ap=h_all[h][:],
                    idxs_ap=idx_sb[:, int(icol0[t, h]):int(icol0[t, h]) + 8 * nblk],
                    num_idxs=nblk * 128, num_idxs_reg=nblk * 128,
                    elem_size=D_OUT, queue_num=qc % NQ)
                qc += 1
                ohs = ohp.tile([128, BMAX * 128], FP8, name="oh" + tag,
                               tag="oh")
                b0 = int(bcol0[t, h])
                nc.sync.dma_start(out=ohs[:, :nblk * 128],
                                  in_=oh[:, b0 * 128:(b0 + nblk) * 128])
                acc = psp.tile([128, D_OUT], F32, name="p" + tag, tag="p")
                for b in range(nblk):
                    nc.tensor.matmul(
                        acc[:], ohs[:, b * 128:(b + 1) * 128],
                        ga[:, b, :], start=(b == 0), stop=(b == nblk - 1))
                return acc, ginst

            # ---- phase 3a: space-0 gathers, accumulate into partials ----
            for t in range(T):
                if NB[t, 0] == 0:
                    continue
                pa, gi = do_space(t, 0, "a")
                na = sum(1 for u in range(t) if NB[u, 0])
                if na < K_EARLY:
                    # let the first few gathers run before the AG-B trigger
                    _add_dep_helper(agb.ins, gi.ins, sync=False,
                                    reason="AG-B after warmup gathers")
                else:
                    # keep the AG-B trigger ahead of the gather flood
                    _add_dep_helper(gi.ins, agb.ins, sync=False,
                                    reason="gather after AG-B trigger")
                nc.vector.tensor_tensor(partial[:, t, :], pa[:],
                                        partial[:, t, :], mybir.AluOpType.add)

            # ---- phase 3c: space-1 gathers + epilogue ----
            # z_scaled = dis*(acc+partial); Exp(scale*z + bias) does the
            # dis-mult and max-shift inside the Scalar activation.
            for t in range(T):
                nb2 = int(NB[t, 1])
                if t % EG == 0:
                    zc = zcp.tile([128, EG, D_OUT], F32, name="zc", tag="zc")
                pos = t % EG
                if nb2:
                    acc, _ = do_space(t, 1, "c")
                    nc.vector.tensor_tensor(zc[:, pos, :], acc[:],
                                            partial[:, t, :],
                                            mybir.AluOpType.add)
                else:
                    nc.vector.tensor_copy(zc[:, pos, :], partial[:, t, :])
                mx = epip.tile([128, 1], F32, name="mx", tag="mx")
                nc.vector.tensor_reduce(mx[:], zc[:, pos, :],
                                        mybir.AxisListType.X,
                                        mybir.AluOpType.max)
                # nm2s = -dis*rowmax  (the Exp shift)
                nc.vector.tensor_scalar(nm2s[:, t:t + 1], mx[:],
                                        dise_sb[:, t:t + 1], -1.0,
                                        mybir.AluOpType.mult,
                                        mybir.AluOpType.mult)
                ex = epip.tile([128, D_OUT], FP8, name="ex", tag="ex")
                nc.scalar.activation(ex[:], zc[:, pos, :],
                                     mybir.ActivationFunctionType.Exp,
                                     bias=nm2s[:, t:t + 1],
                                     scale=dise_sb[:, t:t + 1],
                                     accum_out=sm[:, t:t + 1])
                if pos == EG - 1:
                    g0 = t - (EG - 1)
                    lse = epip.tile([128, EG], F32, name="lse", tag="lse")
                    nc.scalar.activation(lse[:], sm[:, g0:g0 + EG],
                                         mybir.ActivationFunctionType.Ln)
                    qg = epip.tile([128, EG], F32, name="qg", tag="qg")
                    nc.vector.tensor_tensor(qg[:], nm2s[:, g0:g0 + EG], lse[:],
                                            mybir.AluOpType.subtract)
                    for p in range(EG):
                        res = epip.tile([128, D_OUT], F32, name="res",
                                        tag="res")
                        nc.vector.tensor_scalar(res[:], zc[:, p, :],
                                                dise_sb[:, g0 + p:g0 + p + 1],
                                                qg[:, p:p + 1],
                                                mybir.AluOpType.mult,
                                                mybir.AluOpType.add)
                        nc.sync.dma_start(
                            out=out[(g0 + p) * 128:(g0 + p + 1) * 128, :],
                            in_=res[:])

    nc.compile()
    return nc


def kernel(x, edge_index, weight, bias):
    global LAST_RESULTS
    x = np.asarray(x, dtype=np.float32)
    weight = np.asarray(weight, dtype=np.float32)
    bias = np.asarray(bias, dtype=np.float32)

    pp = _preprocess(x, edge_index, weight, bias)
    idx_cols = pp["idx"].shape[2]
    blk_cols = pp["oh"].shape[2] // 128
    nc = _build(pp["NB"], idx_cols, blk_cols)

    in_maps = []
    for c in range(C):
        in_maps.append({
            "xT": np.ascontiguousarray(pp["xT"][c]),
            "w": pp["w"],
            "disg": np.ascontiguousarray(pp["disg"][c]),
            "dise": np.ascontiguousarray(pp["dise"][c]),
            "disi": np.ascontiguousarray(pp["disi"][c]),
            "biasf": pp["bias"],
            "idx": np.ascontiguousarray(pp["idx"][c]),
            "oh": np.ascontiguousarray(pp["oh"][c]),
        })

    res = run_bass_kernel_spmd(nc, in_maps, core_ids=list(range(C)))
    LAST_RESULTS = res

    out = np.empty((N_NODES, D_OUT), np.float32)
    for c in range(C):
        out[c * NLOC:(c + 1) * NLOC] = res.results[c]["out"][:NLOC]
    return out


# revision 13
# speedup vs baseline: 1.0942x; 1.0214x over previous
"""GCN layer (GCNConv + log_softmax) on 8 Trainium2 NeuronCores.

Sharding: nodes row-sharded 8 ways. Each core computes h' = dis * (x @ W)
for its slice (fp8-e3m4 GEMM with W pre-scaled by 64, rescaled in the
Scalar-engine copy), AllGathers h' in two chunks (A: rows 0-3967 fired
mid-GEMM, B: the 2304-row tail fired at GEMM end) and aggregates
messages for its destination slice with dma_gather calls spread over 4
SWDGE queues (parallel Q7 descriptor generation) + host-built fp8
one-hot indicator matmuls on the tensor engine.  Self-loop and bias
terms are pre-folded into the per-tile partial buffer during the GEMM,
so the epilogue is only: z = acc + partial, row-max, then log_softmax
with the dis/shift folded into the Scalar activation's scale/bias
operands.
"""

import numpy as np
import ml_dtypes

import concourse.bass as bass
import concourse.tile as tile
from concourse import bacc, mybir
from concourse.bass import _add_dep_helper
from concourse.bass_utils import run_bass_kernel_spmd

bf16 = ml_dtypes.bfloat16
f8 = ml_dtypes.float8_e3m4
F32 = mybir.dt.float32
BF16 = mybir.dt.bfloat16
FP8 = mybir.dt.float8e3
I16 = mybir.dt.int16

N_NODES = 50000
D_IN = 2048
D_OUT = 512
C = 8                      # cores
NLOC = N_NODES // C        # 6250 real nodes per core
T = 49                     # dst tiles per core
NPAD = T * 128             # 6272 padded rows per core
SW = 512                   # GEMM node-chunk width
NCHUNK = 13                # 12 full chunks + 128-row tail
WSCALE = 64.0              # weight pre-scale so fp8-e3m4 w stays normal
TILE_A = 31                # dst tiles in gather space A
LOCS = (TILE_A * 128, NPAD - TILE_A * 128)     # (3968, 2304) rows per core
EDGE0 = (0, TILE_A * 128)
KT = D_IN // 128           # 16 contraction chunks
DEPTH = 16                 # gather tile pool depth
NQ = 4                     # SWDGE queues (parallel Q7 descriptor gen)
EG = 7                     # epilogue group size (tiles per Ln batch)

LAST_RESULTS = None        # test harness reads exec_time_ns from here


def _wrap_idx(idx):
    """Wrap a [n] index array into the [128, n//16] dma_gather layout."""
    n = idx.shape[0]
    assert n % 16 == 0
    cols = n // 16
    w = np.empty((128, cols), np.int16)
    blk = idx.reshape(cols, 16).T.astype(np.int16)   # [16, cols]
    for g in range(8):
        w[g * 16:(g + 1) * 16, :] = blk
    return w


def _preprocess(x, edge_index, weight, bias):
    src = np.asarray(edge_index[0], dtype=np.int64)
    dst = np.asarray(edge_index[1], dtype=np.int64)

    # degree includes the self loop; self-loop messages are served from the
    # SBUF copy of h' instead of being gathered.
    deg = (np.bincount(dst, minlength=N_NODES) + 1).astype(np.float32)
    dis = 1.0 / np.sqrt(deg)

    sc = src // NLOC
    sr = src % NLOC
    half = (sr >= EDGE0[1]).astype(np.int64)         # gather space 0/1
    base = np.asarray(LOCS)[half]
    g = sc * base + (sr - np.asarray(EDGE0)[half])

    dc = dst // NLOC                   # dst core
    dr = dst % NLOC
    dt = dr // 128                     # dst tile within core
    dl = dr % 128                      # dst row within tile

    order = np.lexsort((g, half, dt, dc))
    g, dc, dt, dl, half = g[order], dc[order], dt[order], dl[order], half[order]

    key = (dc * T + dt) * 2 + half
    counts = np.bincount(key, minlength=C * T * 2).reshape(C, T, 2)
    blocks = -(-counts // 128)                       # ceil div
    NB = blocks.max(axis=0)                          # [T, 2]

    blk_cols = int(NB.sum())
    idx_cols = 8 * blk_cols
    idx_np = np.zeros((C, 128, idx_cols), np.int16)
    oh_np = np.zeros((C, 128, blk_cols * 128), f8)
    dcol = np.arange(128)

    starts = np.zeros(C * T * 2 + 1, np.int64)
    np.cumsum(np.bincount(key, minlength=C * T * 2), out=starts[1:])

    for c in range(C):
        icol = 0
        bcol = 0
        for t in range(T):
            for h in range(2):
                B = int(NB[t, h])
                if B == 0:
                    continue
                k = (c * T + t) * 2 + h
                seg = slice(starts[k], starts[k + 1])
                n = starts[k + 1] - starts[k]
                cap = B * 128
                gi = np.zeros(cap, np.int64)
                gi[:n] = g[seg]
                dv = np.full(cap, -1.0, np.float32)
                dv[:n] = dl[seg]
                idx_np[c, :, icol:icol + 8 * B] = _wrap_idx(gi)
                ohb = (dv.reshape(B, 128)[:, :, None] == dcol[None, None, :])
                oh_np[c, :, bcol * 128:(bcol + B) * 128] = (
                    ohb.transpose(1, 0, 2).reshape(128, B * 128).astype(f8))
                icol += 8 * B
                bcol += B

    w_f8 = np.ascontiguousarray((weight * WSCALE).astype(f8))
    xT = np.zeros((C, D_IN, NPAD), f8)
    disg_np = np.zeros((C, 128, T), np.float32)
    dise_np = np.zeros((C, 128, T), np.float32)
    disi_np = np.zeros((C, 128, T), np.float32)
    for c in range(C):
        xs = x[c * NLOC:(c + 1) * NLOC]
        xT[c, :, :NLOC] = xs.T.astype(f8)
        dl_ = np.pad(dis[c * NLOC:(c + 1) * NLOC], (0, NPAD - NLOC))
        di_ = np.pad(1.0 / dis[c * NLOC:(c + 1) * NLOC], (0, NPAD - NLOC))
        dise_np[c] = dl_.reshape(T, 128).T
        disg_np[c] = dise_np[c] / WSCALE
        disi_np[c] = di_.reshape(T, 128).T

    bias_full = np.tile(np.asarray(bias, np.float32)[None, :], (128, 1))

    return dict(
        NB=NB, idx=idx_np, oh=oh_np, w=w_f8, xT=xT,
        disg=disg_np, dise=dise_np, disi=disi_np,
        bias=np.ascontiguousarray(bias_full),
    )


def _build(NB, idx_cols, blk_cols):
    nc = bacc.Bacc("TRN2", target_bir_lowering=False, debug=False,
                   num_devices=C, num_swdge_queues=NQ)

    xT_t = nc.dram_tensor("xT", [D_IN, NPAD], FP8, kind="ExternalInput")
    w_t = nc.dram_tensor("w", [D_IN, D_OUT], FP8, kind="ExternalInput")
    disg_t = nc.dram_tensor("disg", [128, T], F32, kind="ExternalInput")
    dise_t = nc.dram_tensor("dise", [128, T], F32, kind="ExternalInput")
    disi_t = nc.dram_tensor("disi", [128, T], F32, kind="ExternalInput")
    bias_t = nc.dram_tensor("biasf", [128, D_OUT], F32, kind="ExternalInput")
    idx_t = nc.dram_tensor("idx", [128, idx_cols], I16, kind="ExternalInput")
    oh_t = nc.dram_tensor("oh", [128, blk_cols * 128], FP8,
                          kind="ExternalInput")
    out_t = nc.dram_tensor("out", [NPAD, D_OUT], F32, kind="ExternalOutput")

    xT, w, disg, dise, disi, biasf, idx, oh, out = (
        t.ap() for t in (xT_t, w_t, disg_t, dise_t, disi_t, bias_t, idx_t,
                         oh_t, out_t))

    BMAX = int(NB.max())

    # per-(tile, space) column offsets into idx / oh
    icol0 = np.zeros((T, 2), np.int64)
    bcol0 = np.zeros((T, 2), np.int64)
    ic = bc = 0
    for t in range(T):
        for h in range(2):
            icol0[t, h] = ic
            ic += 8 * int(NB[t, h])
            bcol0[t, h] = bc
            bc += int(NB[t, h])

    with tile.TileContext(nc) as tc:
        with tc.tile_pool(name="const", bufs=1) as constp, \
             tc.tile_pool(name="xk", bufs=4) as xkp, \
             tc.tile_pool(name="gath", bufs=DEPTH) as gp, \
             tc.tile_pool(name="ohp", bufs=6) as ohp, \
             tc.tile_pool(name="epi", bufs=6) as epip, \
             tc.tile_pool(name="zc", bufs=2) as zcp, \
             tc.tile_pool(name="psum", bufs=4, space="PSUM") as psp, \
             tc.tile_pool(name="dram", bufs=1, space="DRAM") as dramp:

            # resident constants
            w_sb = constp.tile([128, KT, D_OUT], FP8)
            for k in range(KT):
                nc.sync.dma_start(out=w_sb[:, k, :], in_=w[k * 128:(k + 1) * 128, :])
            disg_sb = constp.tile([128, T], F32)
            nc.sync.dma_start(out=disg_sb[:], in_=disg[:])
            dise_sb = constp.tile([128, T], F32)
            nc.sync.dma_start(out=dise_sb[:], in_=dise[:])
            disi_sb = constp.tile([128, T], F32)
            nc.sync.dma_start(out=disi_sb[:], in_=disi[:])
            bias_sb = constp.tile([128, D_OUT], F32)
            nc.sync.dma_start(out=bias_sb[:], in_=biasf[:])
            idx_sb = constp.tile([128, idx_cols], I16)
            nc.sync.dma_start(out=idx_sb[:], in_=idx[:])

            hres = constp.tile([128, T, D_OUT], FP8)     # h' kept on-chip
            partial = constp.tile([128, T, D_OUT], FP8)  # running partials
            sm = constp.tile([128, T], F32)              # exp row-sums
            nm2s = constp.tile([128, T], F32)            # -dis*rowmax stash

            h_loc = [dramp.tile([LOCS[h], D_OUT], FP8, name=f"h_loc{h}")
                     for h in range(2)]
            h_all = [dramp.tile([C * LOCS[h], D_OUT], FP8,
                                addr_space="Shared", name=f"h_all{h}")
                     for h in range(2)]

            # ---- phase 1: h' = dis * (x @ W) in fp8, two allgathers ----
            for s in range(NCHUNK):
                sw = SW if s < NCHUNK - 1 else NPAD - (NCHUNK - 1) * SW
                xk = xkp.tile([128, KT, SW], FP8, name="xk", tag="xk")
                for k in range(KT):
                    nc.sync.dma_start(
                        out=xk[:, k, :sw],
                        in_=xT[k * 128:(k + 1) * 128, s * SW:s * SW + sw])
                for t in range(sw // 128):
                    ph = psp.tile([128, D_OUT], F32, name="ph", tag="ph")
                    for k in range(KT):
                        nc.tensor.matmul(
                            ph[:], xk[:, k, t * 128:(t + 1) * 128],
                            w_sb[:, k, :], start=(k == 0), stop=(k == KT - 1))
                    gt = s * (SW // 128) + t
                    # h' = (x@64W) * dis/64 on the Scalar engine
                    nc.scalar.activation(hres[:, gt, :], ph[:],
                                         mybir.ActivationFunctionType.Copy,
                                         scale=disg_sb[:, gt:gt + 1])
                    # pre-fold bias/dis + self term into the partial buffer
                    nc.vector.tensor_scalar(partial[:, gt, :], bias_sb[:],
                                            disi_sb[:, gt:gt + 1], None,
                                            mybir.AluOpType.mult)
                    nc.vector.tensor_tensor(partial[:, gt, :],
                                            partial[:, gt, :], hres[:, gt, :],
                                            mybir.AluOpType.add)
                    r0 = gt * 128
                    h = 0 if gt < TILE_A else 1
                    ro = r0 - EDGE0[h]
                    nc.sync.dma_start(out=h_loc[h][ro:ro + 128, :],
                                      in_=hres[:, gt, :])
                    if gt == TILE_A - 1:
                        nc.gpsimd.collective_compute(
                            "AllGather", mybir.AluOpType.bypass,
                            replica_groups=[list(range(C))],
                            ins=[h_loc[0].opt()], outs=[h_all[0].opt()])
            agb = nc.gpsimd.collective_compute(
                "AllGather", mybir.AluOpType.bypass,
                replica_groups=[list(range(C))],
                ins=[h_loc[1].opt()], outs=[h_all[1].opt()])

            qc = 0  # gather queue round-robin counter
            K_EARLY = 2   # 3a gathers allowed to run before the AG-B trigger

            def do_space(t, h, tag):
                """Gather + one-hot matmuls for (tile t, space h) -> psum."""
                nonlocal qc
                nblk = int(NB[t, h])
                ga = gp.tile([128, BMAX, D_OUT], FP8, name="g" + tag, tag="g")
                ginst = nc.gpsimd.dma_gather(
                    out_ap=ga[:, :nblk, :], in_ap=h_all[h][:],
                    idxs_ap=idx_sb[:, int(icol0[t, h]):int(icol0[t, h]) + 8 * nblk],
                    num_idxs=nblk * 128, num_idxs_reg=nblk * 128,
                    elem_size=D_OUT, queue_num=qc % NQ)
                qc += 1
                ohs = ohp.tile([128, BMAX * 128], FP8, name="oh" + tag,
                               tag="oh")
                b0 = int(bcol0[t, h])
                nc.sync.dma_start(out=ohs[:, :nblk * 128],
                                  in_=oh[:, b0 * 128:(b0 + nblk) * 128])
                acc = psp.tile([128, D_OUT], F32, name="p" + tag, tag="p")
                for b in range(nblk):
                    nc.tensor.matmul(
                        acc[:], ohs[:, b * 128:(b + 1) * 128],
                        ga[:, b, :], start=(b == 0), stop=(b == nblk - 1))
                return acc, ginst

            # ---- phase 3a: space-0 gathers, accumulate into partials ----
            for t in range(T):
                if NB[t, 0] == 0:
                    continue
                pa, gi = do_space(t, 0, "a")
                na = sum(1 for u in range(t) if NB[u, 0])
                if na < K_EARLY:
                    # let the first few gathers run before the AG-B trigger
                    _add_dep_helper(agb.ins, gi.ins, sync=False,
                                    reason="AG-B after warmup gathers")
                else:
                    # keep the AG-B trigger ahead of the gather flood
                    _add_dep_helper(gi.ins, agb.ins, sync=False,
                                    reason="gather after AG-B trigger")
                nc.vector.tensor_tensor(partial[:, t, :], pa[:],
                                        partial[:, t, :], mybir.AluOpType.add)

            # ---- phase 3c: space-1 gathers + epilogue ----
            # z_scaled = dis*(acc+partial); Exp(scale*z + bias) does the
            # dis-mult and max-shift inside the Scalar activation.
            for t in range(T):
                nb2 = int(NB[t, 1])
                if t % EG == 0:
                    zc = zcp.tile([128, EG, D_OUT], F32, name="zc", tag="zc")
                pos = t % EG
                if nb2:
                    acc, _ = do_space(t, 1, "c")
                    nc.vector.tensor_tensor(zc[:, pos, :], acc[:],
                                            partial[:, t, :],
                                            mybir.AluOpType.add)
                else:
                    nc.vector.tensor_copy(zc[:, pos, :], partial[:, t, :])
                mx = epip.tile([128, 1], F32, name="mx", tag="mx")
                nc.vector.tensor_reduce(mx[:], zc[:, pos, :],
                                        mybir.AxisListType.X,
                                        mybir.AluOpType.max)
                # nm2s = -dis*rowmax  (the Exp shift)
                nc.vector.tensor_scalar(nm2s[:, t:t + 1], mx[:],
                                        dise_sb[:, t:t + 1], -1.0,
                                        mybir.AluOpType.mult,
                                        mybir.AluOpType.mult)
                ex = epip.tile([128, D_OUT], FP8, name="ex", tag="ex")
                nc.scalar.activation(ex[:], zc[:, pos, :],
                                     mybir.ActivationFunctionType.Exp,
                                     bias=nm2s[:, t:t + 1],
                                     scale=dise_sb[:, t:t + 1],
                                     accum_out=sm[:, t:t + 1])
                if pos == EG - 1:
                    g0 = t - (EG - 1)
                    lse = epip.tile([128, EG], F32, name="lse", tag="lse")
                    nc.scalar.activation(lse[:], sm[:, g0:g0 + EG],
                                         mybir.ActivationFunctionType.Ln)
                    qg = epip.tile([128, EG], F32, name="qg", tag="qg")
                    nc.vector.tensor_tensor(qg[:], nm2s[:, g0:g0 + EG], lse[:],
                                            mybir.AluOpType.subtract)
                    for p in range(EG):
                        res = epip.tile([128, D_OUT], F32, name="res",
                                        tag="res")
                        nc.vector.tensor_scalar(res[:], zc[:, p, :],
                                                dise_sb[:, g0 + p:g0 + p + 1],
                                                qg[:, p:p + 1],
                                                mybir.AluOpType.mult,
                                                mybir.AluOpType.add)
                        nc.sync.dma_start(
                            out=out[(g0 + p) * 128:(g0 + p + 1) * 128, :],
                            in_=res[:])

    nc.compile()
    return nc


def kernel(x, edge_index, weight, bias):
    global LAST_RESULTS
    x = np.asarray(x, dtype=np.float32)
    weight = np.asarray(weight, dtype=np.float32)
    bias = np.asarray(bias, dtype=np.float32)

    pp = _preprocess(x, edge_index, weight, bias)
    idx_cols = pp["idx"].shape[2]
    blk_cols = pp["oh"].shape[2] // 128
    nc = _build(pp["NB"], idx_cols, blk_cols)

    in_maps = []
    for c in range(C):
        in_maps.append({
            "xT": np.ascontiguousarray(pp["xT"][c]),
            "w": pp["w"],
            "disg": np.ascontiguousarray(pp["disg"][c]),
            "dise": np.ascontiguousarray(pp["dise"][c]),
            "disi": np.ascontiguousarray(pp["disi"][c]),
            "biasf": pp["bias"],
            "idx": np.ascontiguousarray(pp["idx"][c]),
            "oh": np.ascontiguousarray(pp["oh"][c]),
        })

    res = run_bass_kernel_spmd(nc, in_maps, core_ids=list(range(C)))
    LAST_RESULTS = res

    out = np.empty((N_NODES, D_OUT), np.float32)
    for c in range(C):
        out[c * NLOC:(c + 1) * NLOC] = res.results[c]["out"][:NLOC]
    return out
